# revision 1
# baseline (speedup 1.0000x reference)
"""Trainium2 Bass kernel for a dense transformer block (B=4, T=2048, D=1024, H=16).

Sharding: 8 cores = 4 batches x 2 head-halves.  Each core computes LN1
(folded into weights host-side), Q/K/V for its 8 heads over all 2048
tokens, causal attention in feature-major layout (denominator via a
ones-column appended to V), then a row-parallel Wo partial that is
pairwise ReduceScattered so that each core ends up with a 1024-token
half for LN2 + MLP.  All matmuls run in float32r (TF32-like, full PE
rate at free-dim 512).

Host-side prep (numpy only): fold LayerNorm gains/biases into the
projection weights, transpose x to feature-major, tile weights.
"""

import os
import sys

for _p in ("/opt/trn_rl_repo", "/root/.axon_site/_ro/trn_rl_repo"):
    if os.path.isdir(_p) and _p not in sys.path:
        sys.path.append(_p)

import numpy as np

import concourse.bass as bass
import concourse.tile as tile
from concourse import bacc, mybir
from concourse.bass_utils import run_bass_kernel_spmd

AF = mybir.ActivationFunctionType
ALU = mybir.AluOpType
FP32 = mybir.dt.float32
FP32R = mybir.dt.float32r

B, T, D, H = 4, 2048, 1024, 16
HD = D // H          # 64
DFF = 4 * D          # 4096
P = 128
DK = D // P          # 8   D k-tiles
NT = T // 512        # 4   512-token tiles
HC = H // 2          # 8   local heads per core
DQ = HC * HD         # 512 local qkv width
NOT = DQ // P        # 4   local qkv feature tiles (head pairs)
FFT = DFF // P       # 32  DFF tiles
TOWN = T // 2        # 1024 own tokens after ReduceScatter
NTO = TOWN // 512    # 2
EPS = 1e-5
SCALE = 1.0 / 8.0    # 1/sqrt(HD)


def build_program(debug=False, sim_mode=False):
    nc = bacc.Bacc("TRN2", target_bir_lowering=False, debug=False)

    # ---- DRAM I/O ----
    xT = nc.dram_tensor("xT", [D, T], FP32R, kind="ExternalInput")
    xTo = nc.dram_tensor("xTo", [D, TOWN], FP32R, kind="ExternalInput")
    wqk = nc.dram_tensor("wqk", [2, NOT, DK, P, P], FP32R, kind="ExternalInput")
    wv = nc.dram_tensor("wv", [DK, P, DQ], FP32R, kind="ExternalInput")
    wo = nc.dram_tensor("wo", [DQ, D], FP32R, kind="ExternalInput")
    w1 = nc.dram_tensor("w1", [FFT, DK, P, P], FP32R, kind="ExternalInput")
    w2 = nc.dram_tensor("w2", [DK, FFT, P, P], FP32R, kind="ExternalInput")
    cqk = nc.dram_tensor("cqk", [P, 2 * NOT], FP32, kind="ExternalInput")
    cvb = nc.dram_tensor("cvb", [P, DQ], FP32R, kind="ExternalInput")
    bo = nc.dram_tensor("bo", [P, DK], FP32, kind="ExternalInput")
    c1 = nc.dram_tensor("c1", [P, FFT], FP32, kind="ExternalInput")
    b2 = nc.dram_tensor("b2", [P, DK], FP32, kind="ExternalInput")
    masks = nc.dram_tensor("masks", [P, 896], FP32R, kind="ExternalInput")
    out = nc.dram_tensor("out", [DK, P, TOWN], FP32R, kind="ExternalOutput")
    if debug:
        dbg_q = nc.dram_tensor("dbg_q", [P, NOT, 512], FP32R, kind="ExternalOutput")
        dbg_kT = nc.dram_tensor("dbg_kT", [P, NOT, T], FP32R, kind="ExternalOutput")
        dbg_v = nc.dram_tensor("dbg_v", [P, HC, T // P, HD + 1], FP32R, kind="ExternalOutput")
        dbg_rso = nc.dram_tensor("dbg_rso", [2, DK, P, 512], FP32R, kind="ExternalOutput")
        dbg_x2 = nc.dram_tensor("dbg_x2", [DK, P, TOWN], FP32R, kind="ExternalOutput")

    xT_r = xT.rearrange("(k p) t -> p k t", p=P)
    xTo_r = xTo.rearrange("(k p) t -> p k t", p=P)
    wo_r = wo.rearrange("(pt p) o -> p pt o", p=P)
    out_r = out.rearrange("k p t -> p k t")

    with tile.TileContext(nc) as tc:
        with (
            tc.tile_pool(name="small", bufs=1) as small,
            tc.tile_pool(name="psum", bufs=1, space="PSUM") as psum,
            tc.tile_pool(name="dram", bufs=1, space="DRAM") as dram,
        ):
            # ---- persistent small tiles ----
            ones_sb = small.tile([P, 1], FP32R)
            nc.vector.memset(ones_sb.bitcast(FP32), 1.0)
            cqk_sb = small.tile([P, 2 * NOT], FP32)
            nc.sync.dma_start(cqk_sb, cqk[:, :])
            cvb_sb = small.tile([P, DQ], FP32R)
            nc.sync.dma_start(cvb_sb, cvb[:, :])
            bo_sb = small.tile([P, DK], FP32)
            nc.sync.dma_start(bo_sb, bo[:, :])
            c1_sb = small.tile([P, FFT], FP32)
            nc.sync.dma_start(c1_sb, c1[:, :])
            b2_sb = small.tile([P, DK], FP32)
            nc.sync.dma_start(b2_sb, b2[:, :])
            # LN rows (all on partition 0; separate tiles so each can be
            # a matmul operand with base_partition 0)
            rs2_sb = small.tile([1, TOWN], FP32)
            row_a = small.tile([1, 512], FP32)
            row_b = small.tile([1, 512], FP32)
            rsb2_sb = small.tile([P, TOWN], FP32)
            eps_sb = small.tile([1, 1], FP32)
            nc.vector.memset(eps_sb, EPS)

            # DRAM scratch
            x2_d = dram.tile([DK, P, TOWN], FP32R)
            rs_d = dram.tile([T], FP32)
            rs_in = [dram.tile([2, DK, P, 512], FP32R, name=f"rsin{i}", tag=f"rsin{i}") for i in range(2)]
            rs_out = [dram.tile([DK, P, 512], FP32R, name=f"rsout{i}", tag=f"rsout{i}") for i in range(2)]
            x2_r = x2_d.rearrange("k p t -> p k t")

            # ==== phases A-E: software-pipelined over 512-token tiles ====
            # iter tt: A/B(tt) stats+projections; C/D(tt-1) attention+Wo;
            # ReduceScatter + x2 + LN2 stats for token-half (tt-3).
            with (
                tc.tile_pool(name="kv", bufs=1) as kvp,
                tc.tile_pool(name="abc", bufs=1) as abc,
            ):
                kT_sb = kvp.tile([P, NOT, T], FP32R)        # [64*(h%2)+d, h//2, t]
                rs_tok = kvp.tile([P, T // P], FP32)        # rstd, token-major
                v_sb = kvp.tile([P, HC, T // P, HD + 1], FP32R)
                mask_sb = kvp.tile([P, 896], FP32R)         # extended causal mask
                nc.sync.dma_start(mask_sb, masks[:, :])
                for h in range(HC):
                    nc.vector.memset(v_sb[:, h, :, HD:HD + 1].bitcast(FP32), 1.0)

                qcur_t = [None] * NT
                for tt in range(NT + 1):
                    if tt < NT:
                        ts5 = slice(tt * 512, (tt + 1) * 512)
                        # ---- A: load xT tile (per k-tile), LN1 stats ----
                        xt_t = abc.tile([P, DK, 512], FP32R, tag="xt", bufs=2)
                        s_ps = psum.tile([1, 512], FP32, tag="st", bufs=2)
                        q_ps = psum.tile([1, 512], FP32, tag="st", bufs=2)
                        for kt in range(DK):
                            nc.sync.dma_start(xt_t[:, kt, :], xT_r[:, kt, ts5])
                            nc.tensor.matmul(s_ps, ones_sb, xt_t[:, kt, :],
                                             start=(kt == 0), stop=(kt == DK - 1))
                        for kt in range(DK):
                            xsq = abc.tile([P, 512], FP32R, tag="xsq", bufs=1)
                            nc.vector.tensor_mul(xsq, xt_t[:, kt, :], xt_t[:, kt, :])
                            nc.tensor.matmul(q_ps, ones_sb, xsq,
                                             start=(kt == 0), stop=(kt == DK - 1))
                        # rows: mu (temp), var, rstd
                        mu = abc.tile([1, 512], FP32, tag="murow", bufs=1)
                        rs_row = abc.tile([1, 512], FP32, tag="rsrow", bufs=2)
                        nc.vector.tensor_scalar(mu, s_ps, 1.0 / D, None, ALU.mult)
                        nc.vector.tensor_scalar(row_a, q_ps, 1.0 / D, None, ALU.mult)
                        nc.vector.tensor_mul(row_b, mu, mu)
                        nc.vector.tensor_sub(row_a, row_a, row_b)
                        nc.scalar.activation(row_b, row_a, AF.Sqrt, bias=eps_sb)
                        nc.vector.reciprocal(rs_row, row_b)
                        rsb = abc.tile([P, 512], FP32, tag="rsb", bufs=2)
                        nc.gpsimd.partition_broadcast(rsb, rs_row)
                        # token-major rstd via DRAM bounce
                        nc.sync.dma_start(rs_d[ts5], rs_row)
                        nc.sync.dma_start(
                            rs_tok[:, tt * 4:(tt + 1) * 4],
                            rs_d[ts5].rearrange("(a p) -> p a", p=P))

                        # ---- B: q/k projections ----
                        qcur = abc.tile([P, NOT, 512], FP32R, tag="qcur", bufs=2)
                        qcur_t[tt] = qcur
                        for proj in range(2):  # 0=q, 1=k
                            for ot in range(NOT):
                                wblk = abc.tile([P, DK, P], FP32R, tag="wqk", bufs=3)
                                nc.sync.dma_start(
                                    wblk, wqk[proj, ot].rearrange("k p m -> p k m"))
                                pp = psum.tile([P, 512], FP32, tag="mm", bufs=2)
                                for kt in range(DK):
                                    nc.tensor.matmul(pp, wblk[:, kt, :], xt_t[:, kt, :],
                                                     start=(kt == 0),
                                                     stop=(kt == DK - 1))
                                dest = (qcur[:, ot, :] if proj == 0 else kT_sb[:, ot, ts5])
                                tmp = abc.tile([P, 512], FP32R, tag="ptmp", bufs=1)
                                nc.vector.tensor_mul(tmp, pp, rsb)
                                nc.vector.tensor_scalar(
                                    dest, tmp,
                                    cqk_sb[:, proj * NOT + ot:proj * NOT + ot + 1],
                                    None, ALU.add)
                        # ---- B: v projection (token-major) ----
                        for st in range(4):
                            pp = psum.tile([P, 512], FP32, tag="mm", bufs=2)
                            for kt in range(DK):
                                wvt = abc.tile([P, 512], FP32R, tag="wv", bufs=2)
                                nc.sync.dma_start(wvt, wv[kt])
                                nc.tensor.matmul(pp, xt_t[:, kt, st * P:(st + 1) * P],
                                                 wvt, start=(kt == 0),
                                                 stop=(kt == DK - 1))
                            vtmp = abc.tile([P, 512], FP32R, tag="ptmp", bufs=1)
                            nc.vector.tensor_scalar(
                                vtmp, pp, rs_tok[:, tt * 4 + st:tt * 4 + st + 1],
                                None, ALU.mult)
                            nc.vector.tensor_tensor(
                                v_sb[:, :, tt * 4 + st, 0:HD],
                                vtmp.rearrange("p (h e) -> p h e", h=HC),
                                cvb_sb.rearrange("p (h e) -> p h e", h=HC), ALU.add)
                        if debug and tt == 0:
                            nc.sync.dma_start(dbg_q[:, :, :], qcur)

                    if tt >= 1:
                        # ---- C: attention for q-tile qt = tt-1 ----
                        qt = tt - 1
                        qv = qcur_t[qt]
                        nkt = 4 * qt + 4
                        ysb = abc.tile([P, NOT, 512], FP32R, tag="ysb", bufs=1)
                        for pt in range(NOT):
                            y_ps = [psum.tile([HD + 1, 512], FP32, name=f"yps{hb}",
                                              tag="y", bufs=2) for hb in range(2)]
                            for kt in range(nkt):
                                jband = kt - 4 * qt
                                pexp = []
                                for hb in range(2):
                                    hsl = slice(hb * HD, (hb + 1) * HD)
                                    s_ps2 = psum.tile([P, 512], FP32, tag="s", bufs=2)
                                    nc.tensor.matmul(
                                        s_ps2,
                                        kT_sb[hsl, pt, kt * P:(kt + 1) * P],
                                        qv[hsl, pt, :], start=True, stop=True)
                                    pe = abc.tile([P, 512], FP32R, tag="pexp", bufs=3)
                                    nc.scalar.activation(pe, s_ps2, AF.Exp, scale=SCALE)
                                    if jband >= 0:
                                        moff = 384 - P * jband
                                        nc.vector.tensor_mul(
                                            pe, pe, mask_sb[:, moff:moff + 512])
                                    pexp.append(pe)
                                for hb in range(2):
                                    nc.tensor.matmul(
                                        y_ps[hb], v_sb[:, 2 * pt + hb, kt, :], pexp[hb],
                                        start=(kt == 0), stop=(kt == nkt - 1))
                            for hb in range(2):
                                # denominator: reciprocal on its own lane, DMA
                                # the row down to lane 0, gpsimd-broadcast.
                                den = abc.tile([HD + 1, 512], FP32, tag="den", bufs=1)
                                nc.vector.reciprocal(den[HD:HD + 1, :],
                                                     y_ps[hb][HD:HD + 1, :])
                                rec = abc.tile([1, 512], FP32, tag="rec", bufs=1)
                                nc.sync.dma_start(rec, den[HD:HD + 1, :])
                                rb = abc.tile([HD, 512], FP32, tag="rb", bufs=1)
                                nc.gpsimd.partition_broadcast(rb, rec)
                                if hb == 0:
                                    nc.vector.tensor_mul(ysb[0:HD, pt, :],
                                                         y_ps[hb][0:HD, :], rb)
                                else:
                                    yst = abc.tile([HD, 512], FP32R, tag="yst", bufs=2)
                                    nc.vector.tensor_mul(yst, y_ps[hb][0:HD, :], rb)
                                    nc.sync.dma_start(
                                        ysb[HD:2 * HD, pt, :], yst)
                        # ---- D: Wo partials for q-tile qt ----
                        for ot in range(DK):
                            wob = abc.tile([P, NOT, P], FP32R, tag="wob", bufs=2)
                            nc.sync.dma_start(wob, wo_r[:, :, ot * P:(ot + 1) * P])
                            pp = psum.tile([P, 512], FP32, tag="mm", bufs=2)
                            for pt in range(NOT):
                                nc.tensor.matmul(pp, wob[:, pt, :], ysb[:, pt, :],
                                                 start=(pt == 0), stop=(pt == NOT - 1))
                            ast = abc.tile([P, 512], FP32R, tag="ast", bufs=2)
                            nc.scalar.copy(ast, pp)
                            nc.sync.dma_start(rs_in[qt % 2][qt // 2, ot], ast)
                        if qt >= 2:
                            # ---- ReduceScatter + x2 + LN2 stats for half i ----
                            i = qt - 2
                            io5 = slice(i * 512, (i + 1) * 512)
                            if sim_mode:
                                nc.sync.dma_start(rs_out[i][:, :, :], rs_in[i][0])
                            else:
                                nc.gpsimd.collective_compute(
                                    "ReduceScatter", ALU.add,
                                    replica_groups=[[0, 1], [2, 3], [4, 5], [6, 7]],
                                    ins=[rs_in[i].opt()], outs=[rs_out[i].opt()])
                            s2_ps = psum.tile([1, 512], FP32, tag="st", bufs=2)
                            q2_ps = psum.tile([1, 512], FP32, tag="st", bufs=2)
                            for kt in range(DK):
                                att = abc.tile([P, 512], FP32R, tag="att", bufs=1)
                                nc.sync.dma_start(att, rs_out[i][kt])
                                xo_t = abc.tile([P, 512], FP32R, tag="xo", bufs=1)
                                nc.sync.dma_start(xo_t, xTo_r[:, kt, io5])
                                x2st = abc.tile([P, 512], FP32R, tag="x2st", bufs=1)
                                nc.vector.scalar_tensor_tensor(
                                    x2st, att, bo_sb[:, kt:kt + 1], xo_t,
                                    ALU.add, ALU.add)
                                nc.sync.dma_start(x2_r[:, kt, io5], x2st)
                                nc.tensor.matmul(s2_ps, ones_sb, x2st,
                                                 start=(kt == 0), stop=(kt == DK - 1))
                                xsq = abc.tile([P, 512], FP32R, tag="xsq", bufs=1)
                                nc.vector.tensor_mul(xsq, x2st, x2st)
                                nc.tensor.matmul(q2_ps, ones_sb, xsq,
                                                 start=(kt == 0), stop=(kt == DK - 1))
                            mu2 = abc.tile([1, 512], FP32, tag="murow", bufs=1)
                            nc.vector.tensor_scalar(mu2, s2_ps, 1.0 / D, None, ALU.mult)
                            nc.vector.tensor_scalar(row_a, q2_ps, 1.0 / D, None, ALU.mult)
                            nc.vector.tensor_mul(row_b, mu2, mu2)
                            nc.vector.tensor_sub(row_a, row_a, row_b)
                            nc.scalar.activation(row_b, row_a, AF.Sqrt, bias=eps_sb)
                            nc.vector.reciprocal(rs2_sb[0:1, io5], row_b)
                            nc.gpsimd.partition_broadcast(
                                rsb2_sb[:, io5], rs2_sb[0:1, io5])

                if debug:
                    nc.sync.dma_start(dbg_kT[:, :, :], kT_sb)
                    nc.sync.dma_start(dbg_v[:, :, :, :], v_sb)

            if debug:
                for i in range(2):
                    nc.sync.dma_start(dbg_rso[i], rs_out[i])
                nc.sync.dma_start(dbg_x2[:, :, :], x2_d)

            # =========== phases F/G: MLP over 512-token tiles ==========
            with tc.tile_pool(name="fg", bufs=1) as fg:
                for tt in range(NTO):
                    ts5 = slice(tt * 512, (tt + 1) * 512)
                    x2t = fg.tile([P, DK, 512], FP32R, tag="x2t", bufs=2)
                    nc.sync.dma_start(x2t, x2_r[:, :, ts5])
                    x2s_t = fg.tile([P, DK, 512], FP32R, tag="x2s", bufs=1)
                    for kt in range(DK):
                        nc.vector.tensor_mul(x2s_t[:, kt, :], x2t[:, kt, :],
                                             rsb2_sb[:, ts5])
                    m_sb = fg.tile([P, FFT, 512], FP32R, tag="m", bufs=1)
                    for fft in range(FFT):
                        w1b = fg.tile([P, DK, P], FP32R, tag="w1b", bufs=3)
                        nc.sync.dma_start(w1b, w1[fft].rearrange("k p m -> p k m"))
                        pp = psum.tile([P, 512], FP32, tag="mm", bufs=2)
                        for kt in range(DK):
                            nc.tensor.matmul(pp, w1b[:, kt, :], x2s_t[:, kt, :],
                                             start=(kt == 0),
                                             stop=(kt == DK - 1))
                        nc.scalar.activation(m_sb[:, fft, :], pp, AF.Gelu,
                                             bias=c1_sb[:, fft:fft + 1])
                    for ot in range(DK):
                        pp = psum.tile([P, 512], FP32, tag="mm", bufs=2)
                        for half in range(2):
                            w2b = fg.tile([P, FFT // 2, P], FP32R, tag="w2b", bufs=2)
                            nc.gpsimd.dma_start(
                                w2b, w2[ot, half * (FFT // 2):(half + 1) * (FFT // 2)]
                                .rearrange("k p m -> p k m"))
                            for kk in range(FFT // 2):
                                kt = half * (FFT // 2) + kk
                                nc.tensor.matmul(pp, w2b[:, kk, :], m_sb[:, kt, :],
                                                 start=(kt == 0), stop=(kt == FFT - 1))
                        ost = fg.tile([P, 512], FP32R, tag="ost", bufs=2)
                        nc.vector.scalar_tensor_tensor(
                            ost, pp, b2_sb[:, ot:ot + 1], x2t[:, ot, :],
                            ALU.add, ALU.add)
                        nc.sync.dma_start(out_r[:, ot, ts5], ost)

    nc.compile()
    return nc


_NC_CACHE = None


def _get_nc():
    global _NC_CACHE
    if _NC_CACHE is None:
        _NC_CACHE = build_program(debug=bool(int(os.environ.get("KERNEL_DEBUG", "0"))))
    return _NC_CACHE


def prep_in_maps(x, ln1_g, ln1_b, ln2_g, ln2_b, Wq, bq, Wk, bk, Wv, bv,
                 Wo, bo, W1, b1, W2, b2):
    f32 = np.float32
    x = np.asarray(x, f32)
    ln1_g, ln1_b = np.asarray(ln1_g, f32), np.asarray(ln1_b, f32)
    ln2_g, ln2_b = np.asarray(ln2_g, f32), np.asarray(ln2_b, f32)
    Wq, Wk, Wv, Wo = (np.asarray(a, f32) for a in (Wq, Wk, Wv, Wo))
    W1, W2 = np.asarray(W1, f32), np.asarray(W2, f32)
    bq, bk, bv, bo_, b1, b2_ = (np.asarray(a, f32) for a in (bq, bk, bv, bo, b1, b2))

    # fold LN gain AND the mean subtraction (a rank-1 correction) into W:
    # (x - mu) * g @ W  =  x @ (g*W - colsum(g*W)/D)
    Wqg = ln1_g[:, None] * Wq
    Wkg = ln1_g[:, None] * Wk
    Wvg = ln1_g[:, None] * Wv
    Wqg = Wqg - Wqg.sum(0, keepdims=True) / D
    Wkg = Wkg - Wkg.sum(0, keepdims=True) / D
    Wvg = Wvg - Wvg.sum(0, keepdims=True) / D
    cq_full = ln1_b @ Wq + bq
    ck_full = ln1_b @ Wk + bk
    cv_full = ln1_b @ Wv + bv
    W1g = ln2_g[:, None] * W1
    W1g = W1g - W1g.sum(0, keepdims=True) / D
    c1_full = ln2_b @ W1 + b1

    w1_t = np.ascontiguousarray(
        W1g.reshape(DK, P, FFT, P).transpose(2, 0, 1, 3))       # [FFT,DK,P,P]
    w2_t = np.ascontiguousarray(
        W2.reshape(FFT, P, DK, P).transpose(2, 0, 1, 3))        # [DK,FFT,P,P]
    c1_t = np.ascontiguousarray(c1_full.reshape(FFT, P).T)      # [P,FFT]
    b2_t = np.ascontiguousarray(b2_.reshape(DK, P).T)           # [P,DK]
    bo_t = np.ascontiguousarray(bo_.reshape(DK, P).T)           # [P,DK]

    kk = np.arange(P)[:, None]
    cc = np.arange(896)[None, :]
    mk = (kk + 384 <= cc).astype(f32)

    in_maps = []
    for c in range(8):
        b_idx, hh = c // 2, c % 2
        sl = slice(DQ * hh, DQ * hh + DQ)
        xT_c = np.ascontiguousarray(x[b_idx].T)
        wq_c, wk_c = Wqg[:, sl], Wkg[:, sl]
        wqk_t = np.stack([
            np.ascontiguousarray(w.reshape(DK, P, NOT, P).transpose(2, 0, 1, 3))
            for w in (wq_c, wk_c)])                              # [2,NOT,DK,P,P]
        cq_t = cq_full[sl].reshape(NOT, P).T                     # [P,NOT]
        ck_t = ck_full[sl].reshape(NOT, P).T
        in_maps.append({
            "xT": xT_c,
            "xTo": np.ascontiguousarray(xT_c[:, hh * TOWN:(hh + 1) * TOWN]),
            "wqk": wqk_t,
            "wv": np.ascontiguousarray(Wvg[:, sl].reshape(DK, P, DQ)),
            "wo": np.ascontiguousarray(Wo[sl, :]),
            "w1": w1_t,
            "w2": w2_t,
            "cqk": np.ascontiguousarray(np.concatenate([cq_t, ck_t], axis=1)),
            "cvb": np.broadcast_to(cv_full[sl][None, :], (P, DQ)).copy(),
            "bo": bo_t,
            "c1": c1_t,
            "b2": b2_t,
            "masks": mk,
        })
    return in_maps


def assemble_output(results):
    out = np.empty((B, T, D), np.float32)
    for c in range(8):
        b_idx, hh = c // 2, c % 2
        o = results[c]["out"].reshape(D, TOWN)
        out[b_idx, hh * TOWN:(hh + 1) * TOWN, :] = o.T
    return out


def kernel(**inputs):
    nc = _get_nc()
    in_maps = prep_in_maps(**inputs)
    res = run_bass_kernel_spmd(nc, in_maps, list(range(8)))
    return assemble_output(res.results)



# revision 12
# speedup vs baseline: 1.1934x; 1.1934x over previous
"""Trainium2 Bass kernel for a dense transformer block (B=4, T=2048, D=1024, H=16).

Sharding: 8 cores = 4 batches x 2 head-halves.  Each core computes LN1
(folded into weights host-side), Q/K/V for its 8 heads over all 2048
tokens, causal attention in feature-major layout (denominator via a
ones-column appended to V), then a row-parallel Wo partial that is
pairwise ReduceScattered so that each core ends up with a 1024-token
half for LN2 + MLP.

v2 vs baseline:
  - all matmuls in bf16 (same PE rate as fp32r at 512 free, half DMA/SBUF)
  - rstd folded into xs once (no token-major rstd DRAM bounce)
  - attention weights resident in SBUF, loaded once
  - batched softmax denominator (one reciprocal per q-tile)
  - ReduceScatter issued early and overlapped with compute
  - x2 kept in SBUF; xTo preloaded; MLP weights streamed per half
"""

import os
import sys

for _p in ("/opt/trn_rl_repo", "/root/.axon_site/_ro/trn_rl_repo"):
    if os.path.isdir(_p) and _p not in sys.path:
        sys.path.append(_p)

import numpy as np

import concourse.bass as bass
import concourse.tile as tile
from concourse import bacc, mybir
from concourse.bass_utils import run_bass_kernel_spmd

AF = mybir.ActivationFunctionType
ALU = mybir.AluOpType
FP32 = mybir.dt.float32
FP32R = mybir.dt.float32r
BF16 = mybir.dt.bfloat16

B, T, D, H = 4, 2048, 1024, 16
HD = D // H          # 64
DFF = 4 * D          # 4096
P = 128
DK = D // P          # 8   D k-tiles
NT = T // 512        # 4   512-token tiles
HC = H // 2          # 8   local heads per core
DQ = HC * HD         # 512 local qkv width
NOT = DQ // P        # 4   local head-pair tiles
FFT = DFF // P       # 32  DFF tiles
TOWN = T // 2        # 1024 own tokens after ReduceScatter
EPS = 1e-5
SCALE = 1.0 / 8.0    # 1/sqrt(HD)


def build_program(sim_mode=False):
    nc = bacc.Bacc("TRN2", target_bir_lowering=False, debug=False)

    # ---- DRAM I/O ----
    xT = nc.dram_tensor("xT", [D, T], BF16, kind="ExternalInput")
    xTo = nc.dram_tensor("xTo", [D, TOWN], FP32R, kind="ExternalInput")
    wqk = nc.dram_tensor("wqk", [P, 2, NOT, DK, P], BF16, kind="ExternalInput")
    wv = nc.dram_tensor("wv", [P, DK, DQ], BF16, kind="ExternalInput")
    wo = nc.dram_tensor("wo", [P, NOT, D], BF16, kind="ExternalInput")
    w1 = nc.dram_tensor("w1", [P, FFT, DK, P], BF16, kind="ExternalInput")
    w2 = nc.dram_tensor("w2", [P, DK, FFT, P], BF16, kind="ExternalInput")
    cqk = nc.dram_tensor("cqk", [P, 2 * NOT], FP32, kind="ExternalInput")
    cvb = nc.dram_tensor("cvb", [P, DQ], FP32, kind="ExternalInput")
    bo = nc.dram_tensor("bo", [P, DK], FP32, kind="ExternalInput")
    c1 = nc.dram_tensor("c1", [P, FFT], FP32, kind="ExternalInput")
    b2 = nc.dram_tensor("b2", [P, DK], FP32, kind="ExternalInput")
    masks = nc.dram_tensor("masks", [P, 896], BF16, kind="ExternalInput")
    out = nc.dram_tensor("out", [DK, P, TOWN], FP32R, kind="ExternalOutput")

    xT_r = xT.rearrange("(k p) t -> p k t", p=P)
    xTo_r = xTo.rearrange("(k p) t -> p k t", p=P)
    out_r = out.rearrange("k p t -> p k t")

    with tile.TileContext(nc) as tc:
        with (
            tc.tile_pool(name="pers", bufs=1) as pers,
            tc.tile_pool(name="psum", bufs=1, space="PSUM") as psum,
            tc.tile_pool(name="dram", bufs=1, space="DRAM") as dram,
        ):
            # ---- persistent tiles ----
            wqk_sb = pers.tile([P, 2, NOT, DK, P], BF16)
            nc.gpsimd.dma_start(wqk_sb, wqk[:, :, :, :, :])
            wv_sb = pers.tile([P, DK, DQ], BF16)
            nc.gpsimd.dma_start(wv_sb, wv[:, :, :])
            wo_sb = pers.tile([P, NOT, D], BF16)
            nc.gpsimd.dma_start(wo_sb, wo[:, :, :])
            mask_sb = pers.tile([P, 896], BF16)
            nc.sync.dma_start(mask_sb, masks[:, :])
            cqk_sb = pers.tile([P, 2 * NOT], FP32)
            nc.sync.dma_start(cqk_sb, cqk[:, :])
            cvb_sb = pers.tile([P, DQ], FP32)
            nc.sync.dma_start(cvb_sb, cvb[:, :])
            bo_sb = pers.tile([P, DK], FP32)
            nc.sync.dma_start(bo_sb, bo[:, :])
            c1_sb = pers.tile([P, FFT], FP32)
            nc.sync.dma_start(c1_sb, c1[:, :])
            b2_sb = pers.tile([P, DK], FP32)
            nc.sync.dma_start(b2_sb, b2[:, :])
            x2_sb = pers.tile([P, DK, TOWN], FP32R)
            rsb2_sb = pers.tile([P, TOWN], FP32)
            rs2_row = pers.tile([1, TOWN], FP32)

            ones_bf = pers.tile([P, 1], BF16)
            nc.vector.memset(ones_bf, 1.0)
            ones_fr = pers.tile([P, 1], FP32R)
            nc.vector.memset(ones_fr.bitcast(FP32), 1.0)
            eps_sb = pers.tile([1, 1], FP32)
            nc.vector.memset(eps_sb, EPS)

            # DRAM scratch for the ReduceScatter (bf16 partials)
            rs_in = [dram.tile([2, DK, P, 512], BF16, name=f"rsin{i}",
                               tag=f"rsin{i}") for i in range(2)]
            rs_out = [dram.tile([DK, P, 512], BF16, name=f"rsout{i}",
                                tag=f"rsout{i}") for i in range(2)]

            # ==== attention phases, software-pipelined over 512-token tiles ====
            with tc.tile_pool(name="att", bufs=1) as att:
                kT_sb = att.tile([P, NOT, T], BF16)       # [64*hb+d, pair, t]
                v_sb = att.tile([P, HC, T // P, HD + 1], BF16)
                for h in range(HC):
                    nc.vector.memset(v_sb[:, h, :, HD:HD + 1], 1.0)

                qcur_t = [None] * NT
                for tt in range(NT + 1):
                    if tt < NT:
                        ts5 = slice(tt * 512, (tt + 1) * 512)
                        # ---- A: load xT tile, LN1 stats, xs = x_hat ----
                        xt_t = att.tile([P, DK, 512], BF16, tag="xt", bufs=2)
                        nc.sync.dma_start(xt_t, xT_r[:, :, ts5])
                        s_ps = psum.tile([1, 512], FP32, tag="st", bufs=2)
                        q_ps = psum.tile([1, 512], FP32, tag="st", bufs=2)
                        for kt in range(DK):
                            nc.tensor.matmul(s_ps, ones_bf, xt_t[:, kt, :],
                                             start=(kt == 0), stop=(kt == DK - 1))
                        for kt in range(DK):
                            xsq = att.tile([P, 512], BF16, tag="xsq", bufs=2)
                            nc.vector.tensor_mul(xsq, xt_t[:, kt, :], xt_t[:, kt, :])
                            nc.tensor.matmul(q_ps, ones_bf, xsq,
                                             start=(kt == 0), stop=(kt == DK - 1))
                        mu = att.tile([1, 512], FP32, tag="murow", bufs=2)
                        row_a = att.tile([1, 512], FP32, tag="rowa", bufs=2)
                        row_b = att.tile([1, 512], FP32, tag="rowb", bufs=2)
                        rs_row = att.tile([1, 512], FP32, tag="rsrow", bufs=2)
                        nc.vector.tensor_scalar(mu, s_ps, 1.0 / D, None, ALU.mult)
                        nc.vector.tensor_scalar(row_a, q_ps, 1.0 / D, None, ALU.mult)
                        nc.vector.tensor_mul(row_b, mu, mu)
                        nc.vector.tensor_sub(row_a, row_a, row_b)
                        nc.scalar.activation(row_b, row_a, AF.Sqrt, bias=eps_sb)
                        nc.vector.reciprocal(rs_row, row_b)
                        rsb = att.tile([P, 512], FP32, tag="rsb", bufs=2)
                        nc.gpsimd.partition_broadcast(rsb, rs_row)
                        # in-place: xt := xt * rstd  (all stats reads are done)
                        for kt in range(DK):
                            nc.vector.tensor_mul(xt_t[:, kt, :], xt_t[:, kt, :], rsb)
                        xs_t = xt_t

                    if tt >= 1:
                        # ---- C: attention for q-tile qt = tt-1 ----
                        qt = tt - 1
                        qv = qcur_t[qt]
                        nkt = 4 * qt + 4
                        ysb = att.tile([P, NOT, 512], BF16, tag="ysb", bufs=2)
                        den8 = att.tile([8, 512], FP32, tag="den8", bufs=2)
                        rden8 = att.tile([8, 512], FP32, tag="rden8", bufs=1)
                        yc_t = [[None] * 2 for _ in range(NOT)]
                        for pt in range(NOT):
                            y_ps = [psum.tile([HD + 1, 512], FP32, tag="y", bufs=2,
                                              name=f"yps{hb}")
                                    for hb in range(2)]
                            for kt in range(nkt):
                                jband = kt - 4 * qt
                                pexp = []
                                for hb in range(2):
                                    hsl = slice(hb * HD, (hb + 1) * HD)
                                    s_ps2 = psum.tile([P, 512], FP32, tag="s", bufs=2)
                                    nc.tensor.matmul(
                                        s_ps2,
                                        kT_sb[hsl, pt, kt * P:(kt + 1) * P],
                                        qv[hsl, pt, :], start=True, stop=True)
                                    pe = att.tile([P, 512], BF16, tag="pexp", bufs=3)
                                    nc.scalar.activation(pe, s_ps2, AF.Exp, scale=SCALE)
                                    if jband >= 0:
                                        moff = 384 - P * jband
                                        nc.vector.tensor_mul(
                                            pe, pe, mask_sb[:, moff:moff + 512])
                                    pexp.append(pe)
                                for hb in range(2):
                                    nc.tensor.matmul(
                                        y_ps[hb], v_sb[:, 2 * pt + hb, kt, :], pexp[hb],
                                        start=(kt == 0), stop=(kt == nkt - 1))
                            for hb in range(2):
                                j = 2 * pt + hb
                                # den row (psum partition 64) -> sbuf -> den8[j]
                                cpden = att.tile([HD + 1, 512], FP32, tag="cpden",
                                                 bufs=2)
                                nc.scalar.copy(cpden[HD:HD + 1, :],
                                               y_ps[hb][HD:HD + 1, :])
                                nc.sync.dma_start(den8[j:j + 1, :],
                                                  cpden[HD:HD + 1, :])
                                # unnormalized y out of psum (bf16)
                                yc = att.tile([HD, 512], BF16, tag="yc", bufs=8)
                                nc.vector.tensor_scalar(yc, y_ps[hb][0:HD, :],
                                                        1.0, None, ALU.mult)
                                yc_t[pt][hb] = yc
                        nc.vector.reciprocal(rden8, den8)
                        for pt in range(NOT):
                            for hb in range(2):
                                j = 2 * pt + hb
                                stage = att.tile([1, 512], FP32, tag="stage", bufs=2)
                                nc.gpsimd.dma_start(stage, rden8[j:j + 1, :])
                                rb = att.tile([HD, 512], FP32, tag="rb", bufs=2)
                                nc.gpsimd.partition_broadcast(rb, stage)
                                if hb == 0:
                                    nc.vector.tensor_mul(ysb[0:HD, pt, :],
                                                         yc_t[pt][hb], rb)
                                else:
                                    yst = att.tile([HD, 512], BF16, tag="yst", bufs=2)
                                    nc.vector.tensor_mul(yst, yc_t[pt][hb], rb)
                                    nc.sync.dma_start(ysb[HD:2 * HD, pt, :], yst)
                        # ---- D: Wo partials for q-tile qt ----
                        for ot in range(DK):
                            pp = psum.tile([P, 512], FP32, tag="mm", bufs=2)
                            for pt in range(NOT):
                                nc.tensor.matmul(pp, wo_sb[:, pt, ot * P:(ot + 1) * P],
                                                 ysb[:, pt, :],
                                                 start=(pt == 0), stop=(pt == NOT - 1))
                            ast = att.tile([P, 512], BF16, tag="ast", bufs=2)
                            nc.scalar.copy(ast, pp)
                            nc.sync.dma_start(rs_in[qt % 2][qt // 2, ot], ast)
                        if qt >= 2:
                            i = qt - 2
                            if sim_mode:
                                nc.sync.dma_start(rs_out[i][:, :, :], rs_in[i][0])
                            else:
                                nc.gpsimd.collective_compute(
                                    "ReduceScatter", ALU.add,
                                    replica_groups=[[0, 1], [2, 3], [4, 5], [6, 7]],
                                    ins=[rs_in[i].opt()], outs=[rs_out[i].opt()])

                    if tt < NT:
                        # ---- B: q/k/v projections for tile tt ----
                        ts5 = slice(tt * 512, (tt + 1) * 512)
                        qcur = att.tile([P, NOT, 512], BF16, tag="qcur", bufs=2)
                        qcur_t[tt] = qcur
                        for proj in range(2):  # 0=q, 1=k
                            for ot in range(NOT):
                                pp = psum.tile([P, 512], FP32, tag="mm", bufs=2)
                                for kt in range(DK):
                                    nc.tensor.matmul(
                                        pp, wqk_sb[:, proj, ot, kt, :], xs_t[:, kt, :],
                                        start=(kt == 0), stop=(kt == DK - 1))
                                dest = (qcur[:, ot, :] if proj == 0
                                        else kT_sb[:, ot, ts5])
                                nc.vector.tensor_scalar(
                                    dest, pp,
                                    cqk_sb[:, proj * NOT + ot:proj * NOT + ot + 1],
                                    None, ALU.add)
                        for st in range(4):
                            pp = psum.tile([P, 512], FP32, tag="mm", bufs=2)
                            for kt in range(DK):
                                nc.tensor.matmul(
                                    pp, xs_t[:, kt, st * P:(st + 1) * P],
                                    wv_sb[:, kt, :],
                                    start=(kt == 0), stop=(kt == DK - 1))
                            nc.vector.tensor_tensor(
                                v_sb[:, :, tt * 4 + st, 0:HD],
                                pp.rearrange("p (h e) -> p h e", h=HC),
                                cvb_sb.rearrange("p (h e) -> p h e", h=HC), ALU.add)

            # ==== consume halves + MLP ====
            with tc.tile_pool(name="fg", bufs=1) as fg:
                for i in range(2):
                    io5 = slice(i * 512, (i + 1) * 512)
                    # ---- consume: x2 = RS partial + bo + x, LN2 stats ----
                    att_t = fg.tile([P, DK, 512], BF16, tag="att", bufs=2)
                    nc.sync.dma_start(att_t, rs_out[i].rearrange("k p t -> p k t"))
                    xo_t = fg.tile([P, DK, 512], FP32R, tag="xo", bufs=1)
                    nc.sync.dma_start(xo_t, xTo_r[:, :, io5])
                    s2_ps = psum.tile([1, 512], FP32, tag="st", bufs=2)
                    q2_ps = psum.tile([1, 512], FP32, tag="st", bufs=2)
                    for kt in range(DK):
                        nc.vector.scalar_tensor_tensor(
                            x2_sb[:, kt, io5], att_t[:, kt, :], bo_sb[:, kt:kt + 1],
                            xo_t[:, kt, :], ALU.add, ALU.add)
                        nc.tensor.matmul(s2_ps, ones_fr, x2_sb[:, kt, io5],
                                         start=(kt == 0), stop=(kt == DK - 1))
                        xsq2 = fg.tile([P, 512], FP32R, tag="xsq2", bufs=2)
                        nc.vector.tensor_mul(xsq2, x2_sb[:, kt, io5],
                                             x2_sb[:, kt, io5])
                        nc.tensor.matmul(q2_ps, ones_fr, xsq2,
                                         start=(kt == 0), stop=(kt == DK - 1))
                    mu2 = fg.tile([1, 512], FP32, tag="mu2", bufs=2)
                    row2a = fg.tile([1, 512], FP32, tag="row2a", bufs=2)
                    row2b = fg.tile([1, 512], FP32, tag="row2b", bufs=2)
                    nc.vector.tensor_scalar(mu2, s2_ps, 1.0 / D, None, ALU.mult)
                    nc.vector.tensor_scalar(row2a, q2_ps, 1.0 / D, None, ALU.mult)
                    nc.vector.tensor_mul(row2b, mu2, mu2)
                    nc.vector.tensor_sub(row2a, row2a, row2b)
                    nc.scalar.activation(row2b, row2a, AF.Sqrt, bias=eps_sb)
                    nc.vector.reciprocal(rs2_row[0:1, io5], row2b)
                    nc.gpsimd.partition_broadcast(rsb2_sb[:, io5], rs2_row[0:1, io5])

                    # ---- MLP over this 512-token half ----
                    x2s_t = fg.tile([P, DK, 512], BF16, tag="x2s", bufs=2)
                    for kt in range(DK):
                        nc.vector.tensor_mul(x2s_t[:, kt, :], x2_sb[:, kt, io5],
                                             rsb2_sb[:, io5])
                    m_sb = fg.tile([P, FFT, 512], BF16, tag="m", bufs=1)
                    for fft in range(FFT):
                        w1b = fg.tile([P, DK, P], BF16, tag="w1b", bufs=4)
                        nc.gpsimd.dma_start(w1b, w1[:, fft, :, :])
                        pp = psum.tile([P, 512], FP32, tag="mm", bufs=2)
                        for kt in range(DK):
                            nc.tensor.matmul(pp, w1b[:, kt, :], x2s_t[:, kt, :],
                                             start=(kt == 0), stop=(kt == DK - 1))
                        nc.scalar.activation(m_sb[:, fft, :], pp, AF.Gelu,
                                             bias=c1_sb[:, fft:fft + 1])
                    for ot in range(DK):
                        w2b = fg.tile([P, FFT, P], BF16, tag="w2b", bufs=2)
                        nc.gpsimd.dma_start(w2b, w2[:, ot, :, :])
                        pp = psum.tile([P, 512], FP32, tag="mm", bufs=2)
                        for kk in range(FFT):
                            nc.tensor.matmul(pp, w2b[:, kk, :], m_sb[:, kk, :],
                                             start=(kk == 0), stop=(kk == FFT - 1))
                        ost = fg.tile([P, 512], FP32R, tag="ost", bufs=2)
                        nc.vector.scalar_tensor_tensor(
                            ost, pp, b2_sb[:, ot:ot + 1], x2_sb[:, ot, io5],
                            ALU.add, ALU.add)
                        nc.sync.dma_start(out_r[:, ot, io5], ost)

    nc.compile()
    return nc


_NC_CACHE = None


def _get_nc():
    global _NC_CACHE
    if _NC_CACHE is None:
        _NC_CACHE = build_program()
    return _NC_CACHE


def prep_in_maps(x, ln1_g, ln1_b, ln2_g, ln2_b, Wq, bq, Wk, bk, Wv, bv,
                 Wo, bo, W1, b1, W2, b2):
    import ml_dtypes
    bf = ml_dtypes.bfloat16
    f32 = np.float32
    x = np.asarray(x, f32)
    ln1_g, ln1_b = np.asarray(ln1_g, f32), np.asarray(ln1_b, f32)
    ln2_g, ln2_b = np.asarray(ln2_g, f32), np.asarray(ln2_b, f32)
    Wq, Wk, Wv, Wo = (np.asarray(a, f32) for a in (Wq, Wk, Wv, Wo))
    W1, W2 = np.asarray(W1, f32), np.asarray(W2, f32)
    bq, bk, bv, bo_, b1, b2_ = (np.asarray(a, f32) for a in (bq, bk, bv, bo, b1, b2))

    # fold LN gain AND the mean subtraction (a rank-1 correction) into W:
    # (x - mu) * g @ W  =  x @ (g*W - colsum(g*W)/D)
    Wqg = ln1_g[:, None] * Wq
    Wkg = ln1_g[:, None] * Wk
    Wvg = ln1_g[:, None] * Wv
    Wqg = Wqg - Wqg.sum(0, keepdims=True) / D
    Wkg = Wkg - Wkg.sum(0, keepdims=True) / D
    Wvg = Wvg - Wvg.sum(0, keepdims=True) / D
    cq_full = ln1_b @ Wq + bq
    ck_full = ln1_b @ Wk + bk
    cv_full = ln1_b @ Wv + bv
    W1g = ln2_g[:, None] * W1
    W1g = W1g - W1g.sum(0, keepdims=True) / D
    c1_full = ln2_b @ W1 + b1

    w1_t = np.ascontiguousarray(
        W1g.reshape(DK, P, FFT, P).transpose(1, 2, 0, 3)).astype(bf)  # [P,FFT,DK,P]
    w2_t = np.ascontiguousarray(
        W2.reshape(FFT, P, DK, P).transpose(1, 2, 0, 3)).astype(bf)   # [P,DK,FFT,P]
    c1_t = np.ascontiguousarray(c1_full.reshape(FFT, P).T)            # [P,FFT]
    b2_t = np.ascontiguousarray(b2_.reshape(DK, P).T)                 # [P,DK]
    bo_t = np.ascontiguousarray(bo_.reshape(DK, P).T)                 # [P,DK]

    kk = np.arange(P)[:, None]
    cc = np.arange(896)[None, :]
    mk = (kk + 384 <= cc).astype(bf)

    in_maps = []
    for c in range(8):
        b_idx, hh = c // 2, c % 2
        sl = slice(DQ * hh, DQ * hh + DQ)
        xT_c = np.ascontiguousarray(x[b_idx].T)
        wq_c, wk_c = Wqg[:, sl], Wkg[:, sl]
        # [P, 2, NOT, DK, P]: arr[p,proj,ot,kt,m] = W[kt*P+p, ot*P+m]
        wqk_t = np.ascontiguousarray(
            np.stack([w.reshape(DK, P, NOT, P) for w in (wq_c, wk_c)])
            .transpose(2, 0, 3, 1, 4)).astype(bf)
        cq_t = cq_full[sl].reshape(NOT, P).T                     # [P,NOT]
        ck_t = ck_full[sl].reshape(NOT, P).T
        in_maps.append({
            "xT": xT_c.astype(bf),
            "xTo": np.ascontiguousarray(xT_c[:, hh * TOWN:(hh + 1) * TOWN]),
            "wqk": wqk_t,
            "wv": np.ascontiguousarray(
                Wvg[:, sl].reshape(DK, P, DQ).transpose(1, 0, 2)).astype(bf),
            "wo": np.ascontiguousarray(
                Wo[sl, :].reshape(NOT, P, D).transpose(1, 0, 2)).astype(bf),
            "w1": w1_t,
            "w2": w2_t,
            "cqk": np.ascontiguousarray(np.concatenate([cq_t, ck_t], axis=1)),
            "cvb": np.broadcast_to(cv_full[sl][None, :], (P, DQ)).copy(),
            "bo": bo_t,
            "c1": c1_t,
            "b2": b2_t,
            "masks": mk,
        })
    return in_maps


def assemble_output(results):
    out = np.empty((B, T, D), np.float32)
    for c in range(8):
        b_idx, hh = c // 2, c % 2
        o = results[c]["out"].reshape(D, TOWN)
        out[b_idx, hh * TOWN:(hh + 1) * TOWN, :] = o.T
    return out


def kernel(**inputs):
    nc = _get_nc()
    in_maps = prep_in_maps(**inputs)
    res = run_bass_kernel_spmd(nc, in_maps, list(range(8)))
    return assemble_output(res.results)
